# revision 3
# baseline (speedup 1.0000x reference)
"""Masked-loss kernel for nn_MLoss_9715216024200 on 8 Trainium2 NeuronCores.

loss = sum(where(y[...,0]>0.5, (y-x)^2 - a*x^2, 0)) + a*sum(x[...,0]^2)
with x,y f32 (256, 10647, 5); output is a f32 scalar.

Sharding: flatten to cells (5 contiguous values each), pad with 256 zero
cells (neutral: y0=0 -> mask 0, x=0 -> no bg term), reshape to
(8 cores, 128 partitions, 2662 cells), and ship the shards as bf16 --
the loss tolerates bf16 inputs (rel err ~1e-4 << 2e-2) and it halves the
HBM stream to 13310 B/partition/tensor (~19us at the 360 GB/s DMA
roofline, which this kernel saturates).

Per-core math uses mask idempotence (m in {0,1} => m^2 = m):

  sum(m*(d^2 - a*x^2)) = sum((m*y)^2) - 2*sum((m*x) o y) + (1-a)*sum((m*x)^2)

so only TWO masked tensors (my = m*y, mx = m*x) are ever materialized --
in a SINGLE DVE tensor_tensor per tile: x and y live in one SBUF tile
xy=[x|y], and the mask multiplies both halves through an outer-broadcast
AP ([P,2,n] view with the mask's outer stride 0), which keeps the packed
bf16 2x DVE rate.  Work spreads across ALL FIVE engines, each under the
~19us DMA stream:

  Pool+DVE: m5 = bf16(y0 > 0.5) replicated to 5 features (Pool 1.44/elem
            takes the early/middle tiles, DVE 0.52/elem the late ones)
  DVE:      mxy = xy * m5b  (one 2n-elem bf16 tensor_tensor, 0.52/elem)
  ACT:      sum((my)^2) via fused Square+accum  (0.83/elem + ~0.6us/instr)
  PE:       sum(mx o y), sum((mx)^2), sum(x0^2) as Gram diagonals: per
            128-col block, matmul(psA+=mx^T y), matmul(psB+=mx^T x), and
            per 128-cell block matmul(psC+=x0^T x0), all accumulating in
            one PSUM tile (53ns/matmul; the trace of each 128x128 Gram is
            taken on the host from a single staged [P,384] export).

The last LAST_VEC tiles skip PE (cross term via DVE ttr, squares on ACT)
so the Grams close early and the PSUM export overlaps the tail.
Host combines in f64.
"""
import sys

for _p in ('/opt/trn_rl_repo',):
    if _p in sys.path:
        sys.path.remove(_p)
    sys.path.insert(0, _p)

import os as _os
import numpy as np

B, C, F = 256, 10647, 5
THRESH = 0.5
ALPHA = 0.1
N_CORES = 8
P = 128
CELLS = B * C                      # 2,725,632
CELLS_PER_PART = 2662              # 8*128*2662 = 2,725,888
PAD_CELLS = N_CORES * P * CELLS_PER_PART - CELLS   # 256
FD = CELLS_PER_PART * F            # 13310 elems per partition per core

_ts = _os.environ.get('TILE_SIZES', '')
TILE_SIZES = ([int(v) for v in _ts.split(',')] if _ts
              else [128, 256, 512, 512, 512, 384, 256, 102])
assert sum(TILE_SIZES) == CELLS_PER_PART
N_TILES = len(TILE_SIZES)
# tiles whose mask runs on DVE instead of Pool
_md = _os.environ.get('MASK_DVE', '5,6,7')
MASK_DVE = set(int(v) for v in _md.split(',') if v != '')
# tiles whose sum((my)^2) runs on PE instead of ACT
_mp = _os.environ.get('MYSQ_PE', '2')
MYSQ_PE = set(int(v) for v in _mp.split(',') if v != '')
# how many trailing tiles skip PE (cross term on DVE ttr, squares on ACT)
LAST_VEC = int(_os.environ.get('LAST_VEC', '1'))
BUFS = [int(v) for v in _os.environ.get('BUFS', '6,4,4,2').split(',')]

_compiled = None


def _build():
    from contextlib import ExitStack
    import concourse.tile as tile
    from concourse import bacc, mybir

    sqa = float(np.sqrt(ALPHA))

    nc = bacc.Bacc("TRN2", target_bir_lowering=False, debug=False,
                   enable_asserts=True, num_devices=N_CORES)
    bf16 = mybir.dt.bfloat16
    f32 = mybir.dt.float32
    x_d = nc.dram_tensor("x", [P, FD], bf16, kind="ExternalInput").ap()
    y_d = nc.dram_tensor("y", [P, FD], bf16, kind="ExternalInput").ap()
    o_d = nc.dram_tensor("o", [P, 4 * N_TILES], f32, kind="ExternalOutput").ap()
    g_d = nc.dram_tensor("g", [P, 512], f32, kind="ExternalOutput").ap()

    Sq = mybir.ActivationFunctionType.Square
    Alu = mybir.AluOpType

    # psum column ranges: A=cross(mx,y)  B=(mx)^2  C=bg x0^2  D=(my)^2-on-PE
    first_pe = [True, True, True, True]
    n_pe_tiles = N_TILES - LAST_VEC

    with tile.TileContext(nc) as tc, ExitStack() as ctx:
        xyp = ctx.enter_context(tc.tile_pool(name="xy", bufs=BUFS[0]))
        mp_ = ctx.enter_context(tc.tile_pool(name="m", bufs=BUFS[1]))
        wp = ctx.enter_context(tc.tile_pool(name="w", bufs=BUFS[2]))
        sp = ctx.enter_context(tc.tile_pool(name="s", bufs=BUFS[3]))
        ap_ = ctx.enter_context(tc.tile_pool(name="acc", bufs=1))
        pp = ctx.enter_context(tc.psum_pool(name="ps", bufs=1))

        acc = ap_.tile([P, 4 * N_TILES], f32)
        gst = ap_.tile([P, 512], f32)
        ps = pp.tile([P, 512], f32)

        off = 0
        for t, cells in enumerate(TILE_SIZES):
            fd = cells * F
            xy = xyp.tile([P, 2 * fd], bf16, tag="xy")
            xv = xy[:, 0:fd]
            yv = xy[:, fd:2 * fd]
            sl = slice(off, off + fd)
            off += fd
            nc.sync.dma_start(yv, y_d[:, sl])
            nc.sync.dma_start(xv, x_d[:, sl])

            # mask replicated to all 5 features
            m5 = mp_.tile([P, fd], bf16, tag="m5")
            y0b = yv[:, 0::F].unsqueeze(2).broadcast_to((P, cells, F))
            m5_eng = nc.vector if t in MASK_DVE else nc.gpsimd
            m5_eng.tensor_scalar(
                m5[:].rearrange("p (k f) -> p k f", f=F), y0b,
                THRESH, None, op0=Alu.is_gt)

            # mxy = [mx | my] in one bf16 2x tensor_tensor
            mxy = wp.tile([P, 2 * fd], bf16, tag="mxy")
            m5b = m5[:].unsqueeze(1).broadcast_to((P, 2, fd))
            nc.vector.tensor_tensor(
                mxy[:].rearrange("p (k n) -> p k n", k=2),
                xy[:].rearrange("p (k n) -> p k n", k=2),
                m5b, op=Alu.mult)
            mxv = mxy[:, 0:fd]
            myv = mxy[:, fd:2 * fd]

            if t not in MYSQ_PE or t >= n_pe_tiles:
                sq = sp.tile([P, fd], bf16, tag="sq")
                nc.scalar.activation(sq[:], myv, Sq, accum_out=acc[:, t:t + 1])

            if t < n_pe_tiles:
                last_pe_tile = (t == n_pe_tiles - 1)
                nb = (fd + 127) // 128
                for j in range(nb):
                    lo = j * 128
                    w = min(128, fd - lo)
                    is_last = last_pe_tile and (j == nb - 1)
                    nc.tensor.matmul(ps[0:w, 0:w], mxv[:, lo:lo + w],
                                     yv[:, lo:lo + w],
                                     start=first_pe[0], stop=is_last,
                                     skip_group_check=True)
                    first_pe[0] = False
                    nc.tensor.matmul(ps[0:w, 128:128 + w], mxv[:, lo:lo + w],
                                     xv[:, lo:lo + w],
                                     start=first_pe[1], stop=is_last,
                                     skip_group_check=True)
                    first_pe[1] = False
                    if t in MYSQ_PE:
                        nc.tensor.matmul(ps[0:w, 384:384 + w],
                                         myv[:, lo:lo + w], myv[:, lo:lo + w],
                                         start=first_pe[3], stop=is_last,
                                         skip_group_check=True)
                        first_pe[3] = False
                # background: x0 (stride-5 view) Gram over 128-cell blocks
                x0v = xv[:, 0::F]
                nbc = (cells + 127) // 128
                for j in range(nbc):
                    lo = j * 128
                    w = min(128, cells - lo)
                    is_last = last_pe_tile and (j == nbc - 1)
                    nc.tensor.matmul(ps[0:w, 256:256 + w], x0v[:, lo:lo + w],
                                     x0v[:, lo:lo + w],
                                     start=first_pe[2], stop=is_last,
                                     skip_group_check=True)
                    first_pe[2] = False
                if last_pe_tile:
                    # stage Grams to SBUF + export (overlaps the tail tiles)
                    nc.vector.tensor_copy(gst[:], ps[:])
                    nc.scalar.dma_start(g_d, gst[:])
            else:
                # tail tile off PE: cross on DVE ttr, squares + bg on ACT
                cw = sp.tile([P, fd], bf16, tag="cw")
                nc.vector.tensor_tensor_reduce(
                    cw[:], mxv, yv, 1.0, 0.0,
                    op0=Alu.mult, op1=Alu.add,
                    accum_out=acc[:, N_TILES + t:N_TILES + t + 1])
                sq2 = sp.tile([P, fd], bf16, tag="sq2")
                nc.scalar.activation(sq2[:], mxv, Sq,
                                     accum_out=acc[:, 2 * N_TILES + t:
                                                   2 * N_TILES + t + 1])
                sq3 = sp.tile([P, cells], bf16, tag="sq3")
                nc.scalar.activation(sq3[:], xv[:, 0::F], Sq, scale=sqa,
                                     accum_out=acc[:, 3 * N_TILES + t:
                                                   3 * N_TILES + t + 1])

        nc.scalar.dma_start(o_d, acc[:])

    nc.compile()
    return nc


def _shard(a: np.ndarray) -> list[np.ndarray]:
    import ml_dtypes
    flat = a.reshape(-1)
    pad = np.zeros(PAD_CELLS * F, dtype=a.dtype)
    flat = np.concatenate([flat, pad]).astype(ml_dtypes.bfloat16)
    per_core = flat.reshape(N_CORES, P, FD)
    return [np.ascontiguousarray(per_core[i]) for i in range(N_CORES)]


def kernel(x: np.ndarray, y: np.ndarray) -> np.ndarray:
    global _compiled
    if _compiled is None:
        _compiled = _build()
    nc = _compiled

    from concourse.bass_utils import run_bass_kernel_spmd

    xs = _shard(np.asarray(x, dtype=np.float32))
    ys = _shard(np.asarray(y, dtype=np.float32))
    in_maps = [{"x": xs[i], "y": ys[i]} for i in range(N_CORES)]
    res = run_bass_kernel_spmd(nc, in_maps, core_ids=list(range(N_CORES)))

    T = N_TILES
    n_pe_tiles = T - LAST_VEC
    total = np.float64(0.0)
    for r in res.results:
        o = r["o"].astype(np.float64)
        g = r["g"].astype(np.float64)
        trA = np.trace(g[:, 0:128])
        trB = np.trace(g[:, 128:256])
        trC = np.trace(g[:, 256:384])
        trD = np.trace(g[:, 384:512])
        # sum (my)^2: ACT cols for non-PE tiles, psD for MYSQ_PE tiles
        myq = sum(o[:, t].sum() for t in range(T)
                  if t not in MYSQ_PE or t >= n_pe_tiles) + trD
        cross = trA + sum(o[:, T + t].sum() for t in range(n_pe_tiles, T))
        mxq = trB + sum(o[:, 2 * T + t].sum() for t in range(n_pe_tiles, T))
        bg = ALPHA * trC + sum(o[:, 3 * T + t].sum()
                               for t in range(n_pe_tiles, T))
        total += myq - 2.0 * cross + (1.0 - ALPHA) * mxq + bg
    return np.float32(total)
